# revision 5
# baseline (speedup 1.0000x reference)
"""Trainium2 Bass kernel for nn_ConditionalSoftmax (sampled-softmax NLL loss).

Computes, for each batch row b:
    v_c   = vectors[cs[b]]                      # [D]
    h     = relu(v_c @ W1 + b1)                 # [H]
    logit = h @ W2 + b2                         # [V]
    nll_b = logsumexp(logit) - logit[v2s[ws[b]]]

Sharding: data-parallel over batch across 8 NeuronCores (1024 rows/core),
weights replicated.  Per core the dominant work is the [1024,512]@[512,20000]
matmul plus the exp of all 20.5M logits.  The matmul runs in fp8_e4m3 with
the PE's DoubleRow perf mode (K=256 per instruction, 2x bf16 throughput);
W2 is pre-scaled by 32 on the host so its values sit in the fp8 normal
range, and the Exp activation's scale parameter undoes the factor for free.
W2 (fp8, 80KB/partition) stays resident in SBUF.  Logits accumulate in
[128,2000] PSUM tiles (4 banks, double buffered) and are reduced in place
by the ScalarEngine's fused exp+row-sum (accum_out), so the [1024,20000]
logit matrix never touches HBM and the per-instruction activation overhead
is amortized over 2000 columns.  The final log runs as ONE batched Ln over
[128,8] so the Exp/Ln activation tables swap exactly once.  The target
logit takes a separate cheap path: indirect-gather of the needed W2.T rows
(fp32) and a multiply-reduce on the VectorEngine against an fp32 recompute
of h.
"""

import numpy as np
import ml_dtypes

import concourse.bass as bass
import concourse.mybir as mybir
import concourse.tile as tile
from concourse import bacc, bass_utils
from concourse.bass import IndirectOffsetOnAxis, ts

# Problem shapes (hardcoded per contest contract)
N_VOCAB = 50000
V = 20000
D = 300
DP = 384          # D padded to 3*128
NDC = 3           # contraction chunks for D
H = 512
NKG = 2           # DoubleRow contraction groups for H (256 each)
NHC = 4           # 128-row contraction chunks for H
B = 8192
NCORES = 8
BL = B // NCORES  # 1024 rows per core
NBT = BL // 128   # 8 batch tiles of 128 rows
VT = 2000         # vocab tile width (4 PSUM banks)
NVT = V // VT     # 10 vocab tiles
# matmul chunks within a VT tile (cannot cross a 512-f32 PSUM bank boundary)
VCHUNKS = ((0, 512), (512, 512), (1024, 512), (1536, VT - 1536))

W2_SCALE = 32.0   # host pre-scale of W2 into fp8 range; undone by Exp scale

F32 = mybir.dt.float32
BF16 = mybir.dt.bfloat16
FP8 = mybir.dt.float8e4
I32 = mybir.dt.int32
AF = mybir.ActivationFunctionType
OP = mybir.AluOpType
DR = mybir.MatmulPerfMode.DoubleRow

_BUILD_CACHE = {}


def _build(b1_nz: bool, b2_nz: bool):
    key = (b1_nz, b2_nz)
    if key in _BUILD_CACHE:
        return _BUILD_CACHE[key]

    nc = bacc.Bacc(
        "TRN2",
        target_bir_lowering=False,
        debug=False,
        num_devices=NCORES,
        num_swdge_queues=4,
    )

    cs_idx = nc.dram_tensor("cs_idx", [NBT, 128, 1], I32, kind="ExternalInput").ap()
    ws_idx = nc.dram_tensor("ws_idx", [NBT, 128, 1], I32, kind="ExternalInput").ap()
    vectors = nc.dram_tensor("vectors", [N_VOCAB, D], F32, kind="ExternalInput").ap()
    v2s = nc.dram_tensor("v2s", [N_VOCAB, 1], I32, kind="ExternalInput").ap()
    w1 = nc.dram_tensor("w1", [DP, H], BF16, kind="ExternalInput").ap()
    b1c = nc.dram_tensor("b1c", [NHC, 128, 1], F32, kind="ExternalInput").ap()
    # W2 pre-scaled by W2_SCALE, fp8, laid out [p, kg, i, v] with
    # W2s[kg*256 + i*128 + p, v] for the PE DoubleRow weard layout.
    w2q = nc.dram_tensor("w2q", [128, NKG, 2, V], FP8, kind="ExternalInput").ap()
    w2tb = nc.dram_tensor("w2tb", [V, H + 1], F32, kind="ExternalInput").ap()
    if b1_nz:
        b1rep = nc.dram_tensor("b1rep", [128, H], F32, kind="ExternalInput").ap()
    if b2_nz:
        b2rep = nc.dram_tensor("b2rep", [128, V], F32, kind="ExternalInput").ap()
    nll = nc.dram_tensor("nll", [NBT, 128, 1], F32, kind="ExternalOutput").ap()

    with tile.TileContext(nc) as tc:
        with (
            tc.tile_pool(name="consts", bufs=1) as consts,
            tc.tile_pool(name="idx", bufs=8) as idxp,
            tc.tile_pool(name="vc", bufs=4) as vcp,
            tc.tile_pool(name="gw", bufs=4) as gwp,
            tc.tile_pool(name="ps", bufs=2, space="PSUM") as psm,
        ):
            w1sb = consts.tile([128, NDC, H], BF16)
            nc.sync.dma_start(w1sb[:], w1.rearrange("(c p) h -> p c h", p=128))
            b1sb = consts.tile([128, NHC], F32)
            for hc in range(NHC):
                nc.sync.dma_start(b1sb[:, hc : hc + 1], b1c[hc])
            if b1_nz:
                b1rep_sb = consts.tile([128, H], F32)
                nc.sync.dma_start(b1rep_sb[:], b1rep[:])
            if b2_nz:
                b2rep_sb = consts.tile([128, V], F32)
                nc.sync.dma_start(b2rep_sb[:], b2rep[:])

            # Resident fp8 W2, loaded in v-chunks so phase 2 can start on
            # chunk 0 while later chunks stream in.
            w2sb = consts.tile([128, NKG, 2, V], FP8)
            for v in range(NVT):
                nc.sync.dma_start(
                    w2sb[:, :, :, ts(v, VT)], w2q[:, :, :, ts(v, VT)]
                )

            # Long-lived activations
            vcT = consts.tile([128, NDC, BL], BF16)    # v_c^T, d-major
            hT8 = consts.tile([128, NKG, 2, BL], FP8)  # h^T fp8, DoubleRow layout
            hb = consts.tile([128, NBT, H], F32)       # h, batch-major (target dot)
            sums = consts.tile([128, NBT * NVT], F32)  # per-(b,v) exp partial sums
            tdot = consts.tile([128, NBT], F32)        # target logits
            fin = consts.tile([128, 3 * NBT], F32)     # S | lnS | result

            # ---- Phase 1: gather embeddings, transpose, first layer. ----
            cidxs = []
            for t in range(NBT):
                cidx = idxp.tile([128, 1], I32, tag="cidx")
                nc.sync.dma_start(cidx[:], cs_idx[t])
                cidxs.append(cidx)
            for t in range(NBT):
                vc = vcp.tile([128, D], F32, tag="vc")
                nc.gpsimd.indirect_dma_start(
                    out=vc[:],
                    out_offset=None,
                    in_=vectors[:],
                    in_offset=IndirectOffsetOnAxis(ap=cidxs[t][:, :1], axis=0),
                )
                vcb = vcp.tile([128, DP], BF16, tag="vcb")
                nc.vector.memset(vcb[:, D:DP], 0.0)
                nc.vector.tensor_copy(vcb[:, :D], vc[:])
                # SBUF->SBUF XBAR transpose, one [128,128] block per d-chunk
                for c in range(NDC):
                    nc.sync.dma_start(
                        vcT[:, c, ts(t, 128)], vcb[:, ts(c, 128)], transpose=True
                    )

                # h^T (h-major) for the main matmul.  Each hc chunk is an
                # independent accumulation group, and start=True zeroes the
                # whole 2KB PSUM bank, so each group gets its own bank
                # (columns hc*512).
                pst = psm.tile([128, VT], F32, tag="ps")
                for hc in range(NHC):
                    for c in range(NDC):
                        nc.tensor.matmul(
                            pst[:, hc * 512 : hc * 512 + 128],
                            lhsT=w1sb[:, c, ts(hc, 128)],
                            rhs=vcT[:, c, ts(t, 128)],
                            start=(c == 0),
                            stop=(c == NDC - 1),
                        )
                # batch-major h (fp32) for the target-logit dot
                psb = psm.tile([128, VT], F32, tag="ps")
                for c in range(NDC):
                    nc.tensor.matmul(
                        psb[:, :512],
                        lhsT=vcT[:, c, ts(t, 128)],
                        rhs=w1sb[:, c, :],
                        start=(c == 0),
                        stop=(c == NDC - 1),
                    )
                # relu + bias, cast to fp8 DoubleRow layout (DVE)
                for hc in range(NHC):
                    nc.vector.tensor_scalar(
                        out=hT8[:, hc // 2, hc % 2, ts(t, 128)],
                        in0=pst[:, hc * 512 : hc * 512 + 128],
                        scalar1=b1sb[:, hc : hc + 1],
                        scalar2=0.0,
                        op0=OP.add,
                        op1=OP.max,
                    )
                if b1_nz:
                    nc.vector.tensor_add(psb[:, :512], psb[:, :512], b1rep_sb[:])
                nc.vector.tensor_scalar_max(hb[:, t, :], psb[:, :512], 0.0)

            # ---- Phase 1b: target logit path ----
            for t in range(NBT):
                widx = idxp.tile([128, 1], I32, tag="widx")
                nc.sync.dma_start(widx[:], ws_idx[t])
                sidx = idxp.tile([128, 1], I32, tag="sidx")
                nc.gpsimd.indirect_dma_start(
                    out=sidx[:],
                    out_offset=None,
                    in_=v2s[:],
                    in_offset=IndirectOffsetOnAxis(ap=widx[:, :1], axis=0),
                )
                g = gwp.tile([128, H + 1], F32, tag="g")
                nc.gpsimd.indirect_dma_start(
                    out=g[:],
                    out_offset=None,
                    in_=w2tb[:],
                    in_offset=IndirectOffsetOnAxis(ap=sidx[:, :1], axis=0),
                )
                # (tensor_tensor_reduce is broken on this HW path; use 3 ops)
                gscr = gwp.tile([128, H], F32, tag="gscr")
                nc.vector.tensor_mul(gscr[:], hb[:, t, :], g[:, :H])
                gacc = gwp.tile([128, 1], F32, tag="gacc")
                nc.vector.reduce_sum(
                    out=gacc[:], in_=gscr[:], axis=mybir.AxisListType.X
                )
                nc.vector.tensor_add(tdot[:, t : t + 1], gacc[:], g[:, H : H + 1])

            # ---- Phase 2: fp8 DoubleRow logits in PSUM, fused exp+rowsum ----
            for t in range(NBT):
                for v in range(NVT):
                    ps = psm.tile([128, VT], F32, tag="ps")
                    for lo, w in VCHUNKS:
                        for kg in range(NKG):
                            nc.tensor.matmul(
                                ps[:, lo : lo + w],
                                lhsT=hT8[:, kg, :, ts(t, 128)],
                                rhs=w2sb[:, kg, :, v * VT + lo : v * VT + lo + w],
                                start=(kg == 0),
                                stop=(kg == NKG - 1),
                                perf_mode=DR,
                            )
                    if b2_nz:
                        nc.vector.tensor_add(
                            ps[:], ps[:], b2rep_sb[:, ts(v, VT)]
                        )
                    nc.scalar.activation(
                        ps[:], ps[:], AF.Exp,
                        scale=1.0 / W2_SCALE,
                        accum_out=sums[:, t * NVT + v : t * NVT + v + 1],
                    )

            # ---- Phase 3: logsumexp and output.  One batched Ln so the
            # Exp->Ln activation-table swap happens exactly once. ----
            for t in range(NBT):
                nc.vector.reduce_sum(
                    out=fin[:, t : t + 1],
                    in_=sums[:, ts(t, NVT)],
                    axis=mybir.AxisListType.X,
                )
            nc.scalar.activation(fin[:, NBT : 2 * NBT], fin[:, :NBT], AF.Ln)
            nc.vector.tensor_sub(
                fin[:, 2 * NBT : 3 * NBT], fin[:, NBT : 2 * NBT], tdot[:, :NBT]
            )
            for t in range(NBT):
                nc.sync.dma_start(nll[t], fin[:, 2 * NBT + t : 2 * NBT + t + 1])

    nc.compile()
    _BUILD_CACHE[key] = nc
    return nc


def _prep_inputs(ws, cs, vectors, W1, b1, W2, b2, vector_to_support):
    ws = np.asarray(ws)
    cs = np.asarray(cs)
    vectors = np.asarray(vectors, dtype=np.float32)
    W1 = np.asarray(W1, dtype=np.float32)
    b1 = np.asarray(b1, dtype=np.float32)
    W2 = np.asarray(W2, dtype=np.float32)
    b2 = np.asarray(b2, dtype=np.float32)
    v2s = np.asarray(vector_to_support)

    b1_nz = bool(np.any(b1))
    b2_nz = bool(np.any(b2))

    w1p = np.zeros((DP, H), dtype=ml_dtypes.bfloat16)
    w1p[:D] = W1.astype(ml_dtypes.bfloat16)
    # fp8 DoubleRow weird layout: w2q[p, kg, i, v] = (W2*S)[kg*256+i*128+p, v]
    w2s = (W2 * W2_SCALE).astype(ml_dtypes.float8_e4m3)
    w2q = np.ascontiguousarray(
        w2s.reshape(NKG, 2, 128, V).transpose(2, 0, 1, 3)
    )
    w2tb = np.ascontiguousarray(
        np.concatenate([W2.T, b2[:, None]], axis=1).astype(np.float32)
    )
    b1c = np.ascontiguousarray(b1.reshape(NHC, 128, 1))
    v2s2d = np.ascontiguousarray(v2s.astype(np.int32).reshape(N_VOCAB, 1))

    shared = {
        "vectors": np.ascontiguousarray(vectors),
        "v2s": v2s2d,
        "w1": w1p,
        "b1c": b1c,
        "w2q": w2q,
        "w2tb": w2tb,
    }
    if b1_nz:
        shared["b1rep"] = np.ascontiguousarray(
            np.broadcast_to(b1, (128, H)).astype(np.float32)
        )
    if b2_nz:
        shared["b2rep"] = np.ascontiguousarray(
            np.broadcast_to(b2, (128, V)).astype(np.float32)
        )

    in_maps = []
    for c in range(NCORES):
        sl = slice(c * BL, (c + 1) * BL)
        m = dict(shared)
        m["cs_idx"] = np.ascontiguousarray(
            cs[sl].astype(np.int32).reshape(NBT, 128, 1)
        )
        m["ws_idx"] = np.ascontiguousarray(
            ws[sl].astype(np.int32).reshape(NBT, 128, 1)
        )
        in_maps.append(m)
    return in_maps, b1_nz, b2_nz


def run(inputs: dict, trace: bool = False):
    """Run the SPMD kernel. Returns (output [B] fp32, BassKernelResults)."""
    in_maps, b1_nz, b2_nz = _prep_inputs(**inputs)
    nc = _build(b1_nz, b2_nz)
    res = bass_utils.run_bass_kernel_spmd(
        nc, in_maps, core_ids=list(range(NCORES)), trace=trace
    )
    out = np.concatenate(
        [r["nll"].reshape(-1) for r in res.results]
    ).astype(np.float32)
    return out, res


def kernel(**inputs) -> np.ndarray:
    out, _ = run(inputs, trace=False)
    return out


# revision 13
# speedup vs baseline: 1.0985x; 1.0985x over previous
"""Trainium2 Bass kernel for nn_ConditionalSoftmax (sampled-softmax NLL loss).

Computes, for each batch row b:
    v_c   = vectors[cs[b]]                      # [D]
    h     = relu(v_c @ W1 + b1)                 # [H]
    logit = h @ W2 + b2                         # [V]
    nll_b = logsumexp(logit) - logit[v2s[ws[b]]]

Sharding: data-parallel over batch across 8 NeuronCores (1024 rows/core),
weights replicated.  Per core the dominant work is the [1024,512]@[512,20000]
matmul plus the exp of all 20.5M logits.  The matmul runs in fp8_e4m3 with
the PE's DoubleRow perf mode (K=256 per instruction, 2x bf16 throughput);
W2 is pre-scaled by 32 on the host so its values sit in the fp8 normal
range, and the Exp activation's scale parameter undoes the factor for free.
W2 (fp8, 80KB/partition) stays resident in SBUF.  Logits accumulate in
[128,2000] PSUM tiles (4 banks, double buffered) and are reduced in place
by the ScalarEngine's fused exp+row-sum (accum_out), so the [1024,20000]
logit matrix never touches HBM and the per-instruction activation overhead
is amortized over 2000 columns.  The final log runs as ONE batched Ln over
[128,8] so the Exp/Ln activation tables swap exactly once.  The target
logit takes a separate cheap path: indirect-gather of the needed W2.T rows
(fp32) and a multiply-reduce on the VectorEngine against an fp32 recompute
of h.
"""

import numpy as np
import ml_dtypes

import concourse.bass as bass
import concourse.mybir as mybir
import concourse.tile as tile
from concourse import bacc, bass_utils
from concourse.bass import IndirectOffsetOnAxis, ts

# Problem shapes (hardcoded per contest contract)
N_VOCAB = 50000
V = 20000
D = 300
DP = 384          # D padded to 3*128
NDC = 3           # contraction chunks for D
H = 512
NKG = 2           # DoubleRow contraction groups for H (256 each)
NHC = 4           # 128-row contraction chunks for H
B = 8192
NCORES = 8
BL = B // NCORES  # 1024 rows per core
NBT = BL // 128   # 8 batch tiles of 128 rows
VT = 2000         # vocab tile width (4 PSUM banks)
NVT = V // VT     # 10 vocab tiles
# matmul chunks within a VT tile (cannot cross a 512-f32 PSUM bank boundary)
VCHUNKS = ((0, 512), (512, 512), (1024, 512), (1536, VT - 1536))

W2_SCALE = 32.0   # host pre-scale of W2 into fp8 range; undone by Exp scale

F32 = mybir.dt.float32
BF16 = mybir.dt.bfloat16
FP8 = mybir.dt.float8e4
I32 = mybir.dt.int32
AF = mybir.ActivationFunctionType
OP = mybir.AluOpType
DR = mybir.MatmulPerfMode.DoubleRow

_BUILD_CACHE = {}


def _build(b1_nz: bool, b2_nz: bool):
    key = (b1_nz, b2_nz)
    if key in _BUILD_CACHE:
        return _BUILD_CACHE[key]

    nc = bacc.Bacc(
        "TRN2",
        target_bir_lowering=False,
        debug=False,
        num_devices=NCORES,
        num_swdge_queues=4,
    )

    # Index tensors pre-transposed on the host to [128, NBT] so each loads
    # in ONE cheap DMA (contiguous 32B runs per partition).
    cs_idx = nc.dram_tensor("cs_idx", [128, NBT], I32, kind="ExternalInput").ap()
    ws_idx = nc.dram_tensor("ws_idx", [128, NBT], I32, kind="ExternalInput").ap()
    vectors = nc.dram_tensor("vectors", [N_VOCAB, D], F32, kind="ExternalInput").ap()
    v2s = nc.dram_tensor("v2s", [N_VOCAB, 1], I32, kind="ExternalInput").ap()
    w1 = nc.dram_tensor("w1", [DP, H], BF16, kind="ExternalInput").ap()
    b1c = nc.dram_tensor("b1c", [128, NHC], F32, kind="ExternalInput").ap()
    # W2 pre-scaled by W2_SCALE, fp8, laid out [p, v-chunk, kg, i, vt] with
    # W2s[kg*256 + i*128 + p, v*VT + vt] so each v-chunk is one contiguous
    # 8KB run per partition (cheap DMA descriptor generation).
    w2q = nc.dram_tensor(
        "w2q", [128, NVT, NKG, 2, VT], FP8, kind="ExternalInput"
    ).ap()
    w2tb = nc.dram_tensor("w2tb", [V, H + 1], F32, kind="ExternalInput").ap()
    if b1_nz:
        b1rep = nc.dram_tensor("b1rep", [128, H], F32, kind="ExternalInput").ap()
    if b2_nz:
        b2rep = nc.dram_tensor("b2rep", [128, V], F32, kind="ExternalInput").ap()
    nll = nc.dram_tensor("nll", [128, NBT], F32, kind="ExternalOutput").ap()

    with tile.TileContext(nc) as tc:
        with (
            tc.tile_pool(name="consts", bufs=1) as consts,
            tc.tile_pool(name="idx", bufs=8) as idxp,
            tc.tile_pool(name="vc", bufs=4) as vcp,
            tc.tile_pool(name="gw", bufs=4) as gwp,
            tc.tile_pool(name="ps", bufs=2, space="PSUM") as psm,
        ):
            # Index DMAs first: they gate the whole phase-1 chain and the
            # Sync sequencer issues DMAs serially (~600ns each).
            cidx = consts.tile([128, NBT], I32)
            nc.sync.dma_start(cidx[:], cs_idx[:])
            widx = consts.tile([128, NBT], I32)
            nc.sync.dma_start(widx[:], ws_idx[:])
            b1sb = consts.tile([128, NHC], F32)
            nc.sync.dma_start(b1sb[:], b1c[:])
            w1sb = consts.tile([128, NDC, H], BF16)
            nc.sync.dma_start(w1sb[:], w1.rearrange("(c p) h -> p c h", p=128))
            if b1_nz:
                b1rep_sb = consts.tile([128, H], F32)
                nc.sync.dma_start(b1rep_sb[:], b1rep[:])
            if b2_nz:
                b2rep_sb = consts.tile([128, V], F32)
                nc.sync.dma_start(b2rep_sb[:], b2rep[:])

            # Resident fp8 W2, loaded in v-chunks so phase 2 can start on
            # chunk 0 while later chunks stream in.  Issued from the Scalar
            # engine's sequencer (idle in phase 1) so the big transfers
            # don't clog the Sync sequencer in front of phase-1 DMAs.
            w2sb = consts.tile([128, NVT, NKG, 2, VT], FP8)
            for v in range(NVT):
                nc.scalar.dma_start(w2sb[:, v], w2q[:, v])

            # Long-lived activations
            vcT = consts.tile([128, NDC, BL], BF16)    # v_c^T, d-major
            hT8 = consts.tile([128, NKG, 2, BL], FP8)  # h^T fp8, DoubleRow layout
            hb = consts.tile([128, NBT, H], F32)       # h, batch-major (target dot)
            sums = consts.tile([128, NBT * NVT], F32)  # per-(b,v) exp partial sums
            tdot = consts.tile([128, NBT], F32)        # target logits
            fin = consts.tile([128, 3 * NBT], F32)     # S | lnS | result

            # ---- Phase 1: gather embeddings, transpose, first layer. ----
            for t in range(NBT):
                vc = vcp.tile([128, D], F32, tag="vc")
                nc.gpsimd.indirect_dma_start(
                    out=vc[:],
                    out_offset=None,
                    in_=vectors[:],
                    in_offset=IndirectOffsetOnAxis(ap=cidx[:, t : t + 1], axis=0),
                )
                vcb = vcp.tile([128, DP], BF16, tag="vcb")
                nc.vector.memset(vcb[:, D:DP], 0.0)
                nc.vector.tensor_copy(vcb[:, :D], vc[:])
                # SBUF->SBUF XBAR transpose, one [128,128] block per d-chunk
                for c in range(NDC):
                    nc.sync.dma_start(
                        vcT[:, c, ts(t, 128)], vcb[:, ts(c, 128)], transpose=True
                    )

                # h^T (h-major) for the main matmul.  Each hc chunk is an
                # independent accumulation group, and start=True zeroes the
                # whole 2KB PSUM bank, so each group gets its own bank
                # (columns hc*512).
                pst = psm.tile([128, VT], F32, tag="ps")
                for hc in range(NHC):
                    for c in range(NDC):
                        nc.tensor.matmul(
                            pst[:, hc * 512 : hc * 512 + 128],
                            lhsT=w1sb[:, c, ts(hc, 128)],
                            rhs=vcT[:, c, ts(t, 128)],
                            start=(c == 0),
                            stop=(c == NDC - 1),
                        )
                # batch-major h (fp32) for the target-logit dot
                psb = psm.tile([128, VT], F32, tag="ps")
                for c in range(NDC):
                    nc.tensor.matmul(
                        psb[:, :512],
                        lhsT=vcT[:, c, ts(t, 128)],
                        rhs=w1sb[:, c, :],
                        start=(c == 0),
                        stop=(c == NDC - 1),
                    )
                # relu + bias, cast to fp8 DoubleRow layout (DVE)
                for hc in range(NHC):
                    nc.vector.tensor_scalar(
                        out=hT8[:, hc // 2, hc % 2, ts(t, 128)],
                        in0=pst[:, hc * 512 : hc * 512 + 128],
                        scalar1=b1sb[:, hc : hc + 1],
                        scalar2=0.0,
                        op0=OP.add,
                        op1=OP.max,
                    )
                if b1_nz:
                    nc.vector.tensor_add(psb[:, :512], psb[:, :512], b1rep_sb[:])
                nc.vector.tensor_scalar_max(hb[:, t, :], psb[:, :512], 0.0)

            # ---- Phase 1b: target logit path ----
            for t in range(NBT):
                sidx = idxp.tile([128, 1], I32, tag="sidx")
                nc.gpsimd.indirect_dma_start(
                    out=sidx[:],
                    out_offset=None,
                    in_=v2s[:],
                    in_offset=IndirectOffsetOnAxis(ap=widx[:, t : t + 1], axis=0),
                )
                g = gwp.tile([128, H + 1], F32, tag="g")
                nc.gpsimd.indirect_dma_start(
                    out=g[:],
                    out_offset=None,
                    in_=w2tb[:],
                    in_offset=IndirectOffsetOnAxis(ap=sidx[:, :1], axis=0),
                )
                # (tensor_tensor_reduce is broken on this HW path; use 3 ops)
                gscr = gwp.tile([128, H], F32, tag="gscr")
                nc.vector.tensor_mul(gscr[:], hb[:, t, :], g[:, :H])
                gacc = gwp.tile([128, 1], F32, tag="gacc")
                nc.vector.reduce_sum(
                    out=gacc[:], in_=gscr[:], axis=mybir.AxisListType.X
                )
                nc.vector.tensor_add(tdot[:, t : t + 1], gacc[:], g[:, H : H + 1])

            # ---- Phase 2: fp8 DoubleRow logits in PSUM, fused exp+rowsum ----
            for t in range(NBT):
                for v in range(NVT):
                    ps = psm.tile([128, VT], F32, tag="ps")
                    for lo, w in VCHUNKS:
                        for kg in range(NKG):
                            nc.tensor.matmul(
                                ps[:, lo : lo + w],
                                lhsT=hT8[:, kg, :, ts(t, 128)],
                                rhs=w2sb[:, v, kg, :, lo : lo + w],
                                start=(kg == 0),
                                stop=(kg == NKG - 1),
                                perf_mode=DR,
                            )
                    if b2_nz:
                        nc.vector.tensor_add(
                            ps[:], ps[:], b2rep_sb[:, ts(v, VT)]
                        )
                    nc.scalar.activation(
                        ps[:], ps[:], AF.Exp,
                        scale=1.0 / W2_SCALE,
                        accum_out=sums[:, t * NVT + v : t * NVT + v + 1],
                    )

            # ---- Phase 3: logsumexp and output.  One batched Ln so the
            # Exp->Ln activation-table swap happens exactly once. ----
            for t in range(NBT):
                nc.vector.reduce_sum(
                    out=fin[:, t : t + 1],
                    in_=sums[:, ts(t, NVT)],
                    axis=mybir.AxisListType.X,
                )
            nc.scalar.activation(fin[:, NBT : 2 * NBT], fin[:, :NBT], AF.Ln)
            nc.vector.tensor_sub(
                fin[:, 2 * NBT : 3 * NBT], fin[:, NBT : 2 * NBT], tdot[:, :NBT]
            )
            nc.sync.dma_start(nll[:], fin[:, 2 * NBT : 3 * NBT])

    nc.compile()
    _BUILD_CACHE[key] = nc
    return nc


def _prep_inputs(ws, cs, vectors, W1, b1, W2, b2, vector_to_support):
    ws = np.asarray(ws)
    cs = np.asarray(cs)
    vectors = np.asarray(vectors, dtype=np.float32)
    W1 = np.asarray(W1, dtype=np.float32)
    b1 = np.asarray(b1, dtype=np.float32)
    W2 = np.asarray(W2, dtype=np.float32)
    b2 = np.asarray(b2, dtype=np.float32)
    v2s = np.asarray(vector_to_support)

    b1_nz = bool(np.any(b1))
    b2_nz = bool(np.any(b2))

    w1p = np.zeros((DP, H), dtype=ml_dtypes.bfloat16)
    w1p[:D] = W1.astype(ml_dtypes.bfloat16)
    # fp8 DoubleRow layout, v-chunk-major per partition:
    # w2q[p, v, kg, i, vt] = (W2*S)[kg*256 + i*128 + p, v*VT + vt]
    w2s = (W2 * W2_SCALE).astype(ml_dtypes.float8_e4m3)
    w2q = np.ascontiguousarray(
        w2s.reshape(NKG, 2, 128, NVT, VT).transpose(2, 3, 0, 1, 4)
    )
    w2tb = np.ascontiguousarray(
        np.concatenate([W2.T, b2[:, None]], axis=1).astype(np.float32)
    )
    b1c = np.ascontiguousarray(b1.reshape(NHC, 128).T)
    v2s2d = np.ascontiguousarray(v2s.astype(np.int32).reshape(N_VOCAB, 1))

    shared = {
        "vectors": np.ascontiguousarray(vectors),
        "v2s": v2s2d,
        "w1": w1p,
        "b1c": b1c,
        "w2q": w2q,
        "w2tb": w2tb,
    }
    if b1_nz:
        shared["b1rep"] = np.ascontiguousarray(
            np.broadcast_to(b1, (128, H)).astype(np.float32)
        )
    if b2_nz:
        shared["b2rep"] = np.ascontiguousarray(
            np.broadcast_to(b2, (128, V)).astype(np.float32)
        )

    in_maps = []
    for c in range(NCORES):
        sl = slice(c * BL, (c + 1) * BL)
        m = dict(shared)
        # host-transposed to [128, NBT]: [p, t] = idx[t*128 + p]
        m["cs_idx"] = np.ascontiguousarray(
            cs[sl].astype(np.int32).reshape(NBT, 128).T
        )
        m["ws_idx"] = np.ascontiguousarray(
            ws[sl].astype(np.int32).reshape(NBT, 128).T
        )
        in_maps.append(m)
    return in_maps, b1_nz, b2_nz


def run(inputs: dict, trace: bool = False):
    """Run the SPMD kernel. Returns (output [B] fp32, BassKernelResults)."""
    in_maps, b1_nz, b2_nz = _prep_inputs(**inputs)
    nc = _build(b1_nz, b2_nz)
    res = bass_utils.run_bass_kernel_spmd(
        nc, in_maps, core_ids=list(range(NCORES)), trace=trace
    )
    # nll comes back [128, NBT] with [p, t] = row t*128+p
    out = np.concatenate(
        [r["nll"].T.reshape(-1) for r in res.results]
    ).astype(np.float32)
    return out, res


def kernel(**inputs) -> np.ndarray:
    out, _ = run(inputs, trace=False)
    return out
